# revision 11
# baseline (speedup 1.0000x reference)
"""GraphSAGE 2-layer fraud detector on 8 trn2 NeuronCores.

Strategy (dst-partitioned, matmul scatter):
  - Nodes padded to 50176 = 8 cores x 49 blocks x 128. Core c owns dst rows
    [c*6272, (c+1)*6272). Edges sorted by dst on host; each core gets the
    edges targeting its rows, grouped per 128-node dst block, chunked into
    128-edge chunks.
  - Layer 1 per chunk: indirect-DMA gather x[src] rows (512B each), build
    one-hot P[e,d] = (ldst[e]==d) on DVE, PSUM-accumulate P.T @ msg ->
    agg[dst,feat]. Mean via per-partition recip scale, then
    h = relu(agg@W1l.T + x@W1r.T + b1) computed feature-major (hT) via a PE
    transpose of agg.
  - z = h@W2l.T and o = h@W2r.T + b2 computed per block (mean-aggregation
    commutes with the linear map, so layer 2 aggregates the 2-wide z instead
    of the 256-wide h). z is AllGathered across cores (50KB/core); layer 2
    reuses the same chunk tables to gather z[src] rows and the same one-hot
    scatter into agg2[dst,2]. out = recip*agg2 + o.
"""

import time

import numpy as np

import concourse.bass as bass
import concourse.mybir as mybir
import concourse.tile as tile
from concourse import bacc
from concourse.bass_utils import run_bass_kernel_spmd

N = 50000
E = 800000
IN_C = 128
HID = 256
OUT_C = 2
NCORES = 8
P = 128
NB = 49                 # dst blocks per core
ROWS = NB * P           # 6272 rows per core
NP = NCORES * ROWS      # 50176 padded nodes
ZROWS = NCORES * P      # 1024 rows of the allgathered z tensor [1024, 2*NB]

f32 = mybir.dt.float32
i32 = mybir.dt.int32

DEBUG_TAPS = False


def _host_prep(x, edge_index, W1l, b1, W1r, W2l, b2, W2r):
    src = edge_index[0].astype(np.int64)
    dst = edge_index[1].astype(np.int64)
    cnt = np.bincount(dst, minlength=NP)
    recip = (1.0 / np.maximum(cnt, 1)).astype(np.float32)

    order = np.argsort(dst, kind="stable")
    s_src = src[order]
    s_dst = dst[order]

    block_starts = np.searchsorted(s_dst, np.arange(0, NP + P, P))
    cnt_blk = (block_starts[1:] - block_starts[:-1]).reshape(NCORES, NB)
    nb = np.maximum(1, -(-cnt_blk // P)).max(axis=0)  # [NB] chunks per block pos
    C1 = int(nb.sum())

    src_arr = np.full((NCORES, P, C1), N, dtype=np.int32)
    ldst_arr = np.full((NCORES, P, C1), 255, dtype=np.float32)
    col = 0
    for b in range(NB):
        w = int(nb[b])
        for c in range(NCORES):
            bb = c * NB + b
            s, e = int(block_starts[bb]), int(block_starts[bb + 1])
            k = e - s
            ts = np.full(w * P, N, np.int32)
            tl = np.full(w * P, 255, np.float32)
            ts[:k] = s_src[s:e]
            tl[:k] = s_dst[s:e] - bb * P
            src_arr[c, :, col:col + w] = ts.reshape(w, P).T
            ldst_arr[c, :, col:col + w] = tl.reshape(w, P).T
        col += w

    # layer-2 gathers the same edges from z_full, whose row layout is
    # [core, p, b]: node n lives at flat row (n//6272)*6272 + (n%128)*49
    # + ((n%6272)//128). Pad entries point at node N; their one-hot column
    # is zero so the gathered value never contributes.
    sa = src_arr.astype(np.int64)
    qsrc_arr = ((sa // ROWS) * ROWS + (sa % P) * NB
                + (sa % ROWS) // P).astype(np.int32)

    x_pad = np.zeros((NP + 1, IN_C), np.float32)
    x_pad[:N] = x
    W1lT = np.ascontiguousarray(W1l.T.astype(np.float32))   # [128, 256]
    W1rT = np.ascontiguousarray(W1r.T.astype(np.float32))
    Wzo = np.zeros((P, 8), np.float32)
    for j in range(2):
        Wzo[:, 4 * j:4 * j + 2] = W2l.T[j * P:(j + 1) * P, :]
        Wzo[:, 4 * j + 2:4 * j + 4] = W2r.T[j * P:(j + 1) * P, :]
    b1p = np.ascontiguousarray(np.asarray(b1).reshape(2, P).T.astype(np.float32))
    b2b = np.tile(np.asarray(b2).reshape(1, 2), (P, 1)).astype(np.float32)
    recip_c = recip.reshape(NCORES, NB, P).transpose(0, 2, 1).copy()  # [c,P,NB]
    iota = np.tile(np.arange(P, dtype=np.float32)[None, :], (P, 1))
    ident = np.eye(P, dtype=np.float32)

    in_maps = []
    for c in range(NCORES):
        xT_own = np.ascontiguousarray(
            x_pad[c * ROWS:(c + 1) * ROWS, :].T
        )  # [128, 6272]
        in_maps.append({
            "x_pad": x_pad,
            "src": np.ascontiguousarray(src_arr[c]),
            "ldst": np.ascontiguousarray(ldst_arr[c]),
            "qsrc": np.ascontiguousarray(qsrc_arr[c]),
            "xT_own": xT_own,
            "W1lT": W1lT,
            "W1rT": W1rT,
            "Wzo": Wzo,
            "b1p": b1p,
            "b2b": b2b,
            "recip": np.ascontiguousarray(recip_c[c]),
            "iota": iota,
            "ident": ident,
        })
    return in_maps, [int(v) for v in nb]


def _build(nb):
    C1 = sum(nb)
    nc = bacc.Bacc(None, target_bir_lowering=False, debug=False)

    x_pad_d = nc.dram_tensor("x_pad", [NP + 1, IN_C], f32, kind="ExternalInput")
    src_d = nc.dram_tensor("src", [P, C1], i32, kind="ExternalInput")
    ldst_d = nc.dram_tensor("ldst", [P, C1], f32, kind="ExternalInput")
    qsrc_d = nc.dram_tensor("qsrc", [P, C1], i32, kind="ExternalInput")
    xT_d = nc.dram_tensor("xT_own", [P, ROWS], f32, kind="ExternalInput")
    W1lT_d = nc.dram_tensor("W1lT", [P, HID], f32, kind="ExternalInput")
    W1rT_d = nc.dram_tensor("W1rT", [P, HID], f32, kind="ExternalInput")
    Wzo_d = nc.dram_tensor("Wzo", [P, 8], f32, kind="ExternalInput")
    b1p_d = nc.dram_tensor("b1p", [P, 2], f32, kind="ExternalInput")
    b2b_d = nc.dram_tensor("b2b", [P, 2], f32, kind="ExternalInput")
    recip_d = nc.dram_tensor("recip", [P, NB], f32, kind="ExternalInput")
    iota_d = nc.dram_tensor("iota", [P, P], f32, kind="ExternalInput")
    ident_d = nc.dram_tensor("ident", [P, P], f32, kind="ExternalInput")
    out_d = nc.dram_tensor("out", [P, 2 * NB], f32, kind="ExternalOutput")
    if DEBUG_TAPS:
        dbg_msg_d = nc.dram_tensor("dbg_msg", [P, nb[0] * P], f32,
                                   kind="ExternalOutput")
        dbg_aggm_d = nc.dram_tensor("dbg_aggm", [P, P], f32,
                                    kind="ExternalOutput")
        dbg_h0_d = nc.dram_tensor("dbg_h0", [P, P], f32, kind="ExternalOutput")
        dbg_z_d = nc.dram_tensor("dbg_z", [P, 2 * NB], f32,
                                 kind="ExternalOutput")

    with tile.TileContext(nc) as tc:
        with (
            tc.tile_pool(name="big", bufs=1) as big,
            tc.tile_pool(name="lp", bufs=4) as lp,
            tc.tile_pool(name="pp", bufs=2, space="PSUM") as pp,
            tc.tile_pool(name="dram", bufs=1, space="DRAM") as dp,
        ):
            def load(d, shape, dt, tag):
                t = big.tile(shape, dt, tag=tag)
                nc.sync.dma_start(out=t[:], in_=d[:, :])
                return t

            src_sb = load(src_d, [P, C1], i32, "src")
            ldst_sb = load(ldst_d, [P, C1], f32, "ldst")
            qsrc_sb = load(qsrc_d, [P, C1], i32, "qsrc")
            xT_sb = load(xT_d, [P, ROWS], f32, "xT")
            W1lT_sb = load(W1lT_d, [P, HID], f32, "w1l")
            W1rT_sb = load(W1rT_d, [P, HID], f32, "w1r")
            Wzo_sb = load(Wzo_d, [P, 8], f32, "wzo")
            b1_sb = load(b1p_d, [P, 2], f32, "b1")
            b2_sb = load(b2b_d, [P, 2], f32, "b2")
            recip_sb = load(recip_d, [P, NB], f32, "recip")
            iota_sb = load(iota_d, [P, P], f32, "iota")
            ident_sb = load(ident_d, [P, P], f32, "ident")

            hT = [
                big.tile([P, ROWS], f32, tag=f"hT{j}", name=f"hT{j}")
                for j in range(2)
            ]
            z_sb = big.tile([P, 2 * NB], f32, tag="z")
            o_sb = big.tile([P, 2 * NB], f32, tag="o")
            out_sb = big.tile([P, 2 * NB], f32, tag="outs")

            col = 0
            for b in range(NB):
                w = nb[b]
                pagg = pp.tile([P, P], f32, tag="agg")
                for k in range(w):
                    msg = lp.tile([P, P], f32, tag="msg")
                    nc.gpsimd.indirect_dma_start(
                        out=msg[:],
                        out_offset=None,
                        in_=x_pad_d[:, :],
                        in_offset=bass.IndirectOffsetOnAxis(
                            ap=src_sb[:, col + k:col + k + 1], axis=0
                        ),
                    )
                    if DEBUG_TAPS and b == 0:
                        nc.sync.dma_start(
                            out=dbg_msg_d[:, k * P:(k + 1) * P], in_=msg[:]
                        )
                    Pt = lp.tile([P, P], f32, tag="P")
                    nc.vector.tensor_scalar(
                        out=Pt[:], in0=iota_sb[:],
                        scalar1=ldst_sb[:, col + k:col + k + 1], scalar2=None,
                        op0=mybir.AluOpType.is_equal,
                    )
                    nc.tensor.matmul(
                        out=pagg[:], lhsT=Pt[:], rhs=msg[:],
                        start=(k == 0), stop=(k == w - 1),
                    )
                aggm = lp.tile([P, P], f32, tag="aggm")
                nc.vector.tensor_scalar(
                    out=aggm[:], in0=pagg[:], scalar1=recip_sb[:, b:b + 1],
                    scalar2=None, op0=mybir.AluOpType.mult,
                )
                if DEBUG_TAPS and b == 0:
                    nc.sync.dma_start(out=dbg_aggm_d[:, :], in_=aggm[:])
                ptr = pp.tile([P, P], f32, tag="tr")
                nc.tensor.transpose(out=ptr[:], in_=aggm[:], identity=ident_sb[:])
                aggmT = lp.tile([P, P], f32, tag="aggmT")
                nc.vector.tensor_copy(out=aggmT[:], in_=ptr[:])
                for j in range(2):
                    ph = pp.tile([P, P], f32, tag="h")
                    nc.tensor.matmul(
                        out=ph[:], lhsT=W1lT_sb[:, j * P:(j + 1) * P],
                        rhs=aggmT[:], start=True, stop=False,
                    )
                    nc.tensor.matmul(
                        out=ph[:], lhsT=W1rT_sb[:, j * P:(j + 1) * P],
                        rhs=xT_sb[:, b * P:(b + 1) * P], start=False, stop=True,
                    )
                    nc.scalar.activation(
                        out=hT[j][:, b * P:(b + 1) * P], in_=ph[:],
                        func=mybir.ActivationFunctionType.Relu,
                        bias=b1_sb[:, j:j + 1],
                    )
                if DEBUG_TAPS and b == 0:
                    nc.sync.dma_start(out=dbg_h0_d[:, :], in_=hT[0][:, 0:P])
                pzo = pp.tile([P, 4], f32, tag="zo")
                for j in range(2):
                    nc.tensor.matmul(
                        out=pzo[:], lhsT=hT[j][:, b * P:(b + 1) * P],
                        rhs=Wzo_sb[:, 4 * j:4 * j + 4],
                        start=(j == 0), stop=(j == 1),
                    )
                nc.vector.tensor_copy(out=z_sb[:, 2 * b:2 * b + 2], in_=pzo[:, 0:2])
                nc.vector.tensor_tensor(
                    out=o_sb[:, 2 * b:2 * b + 2], in0=pzo[:, 2:4], in1=b2_sb[:],
                    op=mybir.AluOpType.add,
                )
                col += w

            if DEBUG_TAPS:
                nc.sync.dma_start(out=dbg_z_d[:, :], in_=z_sb[:])

            # z -> DRAM, allgather
            z_own = dp.tile([P, 2 * NB], f32, tag="zown")
            nc.sync.dma_start(out=z_own[:], in_=z_sb[:])
            z_full = dp.tile([ZROWS, 2 * NB], f32, tag="zfull")
            nc.gpsimd.collective_compute(
                "AllGather",
                mybir.AluOpType.bypass,
                replica_groups=[list(range(NCORES))],
                ins=[z_own[:, :]],
                outs=[z_full[:, :]],
            )
            z_rows = z_full[:, :].rearrange("a (r f) -> (a r) f", f=2)

            col = 0
            for b in range(NB):
                w = nb[b]
                pa2 = pp.tile([P, 2], f32, tag="agg", name="pa2")
                for k in range(w):
                    zg = lp.tile([P, 2], f32, tag="zg")
                    nc.gpsimd.indirect_dma_start(
                        out=zg[:],
                        out_offset=None,
                        in_=z_rows,
                        in_offset=bass.IndirectOffsetOnAxis(
                            ap=qsrc_sb[:, col + k:col + k + 1], axis=0
                        ),
                    )
                    P2 = lp.tile([P, P], f32, tag="P2")
                    nc.vector.tensor_scalar(
                        out=P2[:], in0=iota_sb[:],
                        scalar1=ldst_sb[:, col + k:col + k + 1], scalar2=None,
                        op0=mybir.AluOpType.is_equal,
                    )
                    nc.tensor.matmul(
                        out=pa2[:], lhsT=P2[:], rhs=zg[:],
                        start=(k == 0), stop=(k == w - 1),
                    )
                red2 = lp.tile([P, 2], f32, tag="red2")
                nc.vector.tensor_scalar(
                    out=red2[:], in0=pa2[:], scalar1=recip_sb[:, b:b + 1],
                    scalar2=None, op0=mybir.AluOpType.mult,
                )
                nc.vector.tensor_tensor(
                    out=out_sb[:, 2 * b:2 * b + 2], in0=red2[:],
                    in1=o_sb[:, 2 * b:2 * b + 2], op=mybir.AluOpType.add,
                )
                col += w

            nc.sync.dma_start(out=out_d[:, :], in_=out_sb[:])
    nc.compile()
    return nc


def _run(inputs, repeat=1):
    in_maps, nb = _host_prep(**inputs)
    nc = _build(nb)
    best = None
    for _ in range(repeat):
        t0 = time.perf_counter()
        res = run_bass_kernel_spmd(
            nc, [dict(m) for m in in_maps], core_ids=list(range(NCORES))
        )
        dt = time.perf_counter() - t0
        best = dt if best is None else min(best, dt)
    outs = []
    for c in range(NCORES):
        a = res.results[c]["out"]  # [128, 98]
        outs.append(a.reshape(P, NB, 2).transpose(1, 0, 2).reshape(ROWS, 2))
    full = np.concatenate(outs, axis=0)[:N]
    return full.astype(np.float32), best


def kernel(**inputs):
    out, _ = _run(inputs, repeat=1)
    return out
